# revision 12
# baseline (speedup 1.0000x reference)
"""Trainium2 Bass kernel for nn_DSnetwork (gnn_message_passing).

Reference computation (S=131072 subgraphs, G=4096 graphs, N=2M nodes, D=128):
  h_sub  = segment_mean(h_node, subgraph_batch, S)            # [S,128]
  2x DS layers:
    x1 = h_sub @ W + b
    x2 = segment_mean(h_sub, subgraph_idx_batch, G) @ Ws + bs
    h_sub = elu(x1 + x2[subgraph_idx_batch])
  h_graph = segment_mean(h_sub, subgraph_idx_batch, G)
  out = relu(h_graph @ Wf1 + bf1) @ Wf2 + bf2                 # [G,10]

Distribution: data-parallel over contiguous graph ranges (512 graphs per
core, 8 cores).  Indices are sorted, so each core owns contiguous slices
of subgraphs and nodes.  Segment sums run on TensorE as one-hot matmuls
(one-hots built on VectorE from host-precomputed relative ids); the
graph->subgraph broadcast is a transposed-one-hot matmul accumulated
directly into the x1 PSUM tile.  Matmuls are bf16 with fp32 PSUM
accumulation; mean scaling is exact fp32 on ScalarE.

Host-side work is pure index preprocessing and data staging: sharding,
padded placement of subgraphs/nodes into static tiles, relative one-hot
ids, 1/count scale vectors, and dtype casts.
"""

from dataclasses import dataclass

import ml_dtypes
import numpy as np

BF16 = ml_dtypes.bfloat16
P = 128


@dataclass(frozen=True)
class Cfg:
    D: int = 128          # node feature dim
    C: int = 128          # hidden dim
    NCORES: int = 8
    G_SH: int = 512       # graphs per core
    NGC: int = 4          # graph chunks of 128 graphs per core
    T2: int = 34          # seg tiles (128 segs) per graph chunk, padded
    W: int = 128          # phase-1 one-hot window (segs per psum chunk)
    T1: int = 18          # node tile slots (128 nodes) per W-seg chunk
    SWATH: int = 8        # seg tiles per elu swath (ragged tail ok)

    @property
    def NS(self):         # seg tiles per core (padded axis)
        return self.NGC * self.T2

    @property
    def SMAXP(self):      # padded segs per core
        return self.NS * P

    @property
    def NCH(self):        # phase-1 chunks per core
        return self.SMAXP // self.W

    @property
    def NSLOT(self):      # node tile slots per core
        return self.NCH * self.T1


FULL = Cfg()

# ---------------------------------------------------------------------------
# host-side planner: shard + metadata layout
# ---------------------------------------------------------------------------


def _plan_core(cfg, core, h_node_bf16, sb, sib, seg_cnt, g_cnt):
    g0 = core * cfg.G_SH
    W = cfg.W

    hp = np.zeros((cfg.NSLOT * P, cfg.D), dtype=BF16)
    rel = np.full((P, cfg.NSLOT), -1.0, dtype=BF16)
    invs = np.zeros((P, cfg.NS), dtype=np.float32)
    rel2 = np.full((P, cfg.NS), -1.0, dtype=BF16)
    invg = np.zeros((P, cfg.NGC), dtype=np.float32)

    # chunk-local graph id per padded seg (-1 pad; fits bf16 exactly)
    gid_pad = np.full(cfg.SMAXP, -1.0, dtype=np.float64)

    for gc in range(cfg.NGC):
        glo = g0 + gc * P
        ghi = glo + P
        a = int(np.searchsorted(sib, glo))
        b = int(np.searchsorted(sib, ghi))
        nseg = b - a
        assert nseg <= cfg.T2 * P, f"T2 too small: {nseg} > {cfg.T2 * P}"
        base_tile = gc * cfg.T2          # first seg tile of this graph chunk

        gl = (sib[a:b] - glo).astype(np.int64)          # in [0,128)
        pad_pos = base_tile * P
        gid_pad[pad_pos:pad_pos + nseg] = gl

        r2 = np.full(cfg.T2 * P, -1.0, dtype=BF16)
        r2[:nseg] = gl.astype(BF16)
        rel2[:, base_tile:base_tile + cfg.T2] = r2.reshape(cfg.T2, P).T

        ivs = np.zeros(cfg.T2 * P, dtype=np.float32)
        ivs[:nseg] = 1.0 / np.maximum(seg_cnt[a:b], 1).astype(np.float32)
        invs[:, base_tile:base_tile + cfg.T2] = ivs.reshape(cfg.T2, P).T

        invg[:, gc] = 1.0 / np.maximum(g_cnt[glo:ghi], 1).astype(np.float32)

        # node packing: phase-1 chunks of W padded segs
        seg_starts = np.searchsorted(sb, np.arange(a, b + 1))
        nch_per_gc = cfg.T2 * P // W
        for cc in range(nch_per_gc):
            c = (base_tile * P) // W + cc            # global chunk index
            slo = cc * W
            shi = min(slo + W, nseg)
            if slo >= nseg:
                continue
            nlo = int(seg_starts[slo])
            nhi = int(seg_starts[shi])
            nn = nhi - nlo
            assert nn <= cfg.T1 * P, f"T1 too small: {nn} > {cfg.T1 * P}"
            if nn == 0:
                continue
            dst = c * cfg.T1 * P
            hp[dst:dst + nn] = h_node_bf16[nlo:nhi]
            rr = (sb[nlo:nhi] - (a + slo)).astype(BF16)
            rfull = np.full(cfg.T1 * P, -1.0, dtype=BF16)
            rfull[:nn] = rr
            rel[:, c * cfg.T1:(c + 1) * cfg.T1] = rfull.reshape(cfg.T1, P).T

    # device layout: hp_dram[c, p, t*D+d] = node row (c*T1*P + t*P + p)
    hp_dev = np.ascontiguousarray(
        hp.reshape(cfg.NCH, cfg.T1, P, cfg.D).transpose(0, 2, 1, 3)
    ).reshape(cfg.NCH, P, cfg.T1 * cfg.D)

    gidb = np.broadcast_to(gid_pad.astype(BF16), (P, cfg.SMAXP)).copy()

    return {
        "hp": hp_dev,
        "rel": rel,
        "invs": invs,
        "rel2": rel2,
        "invg": invg,
        "gidb": gidb,
    }


def plan(cfg, h_node, sb, sib):
    sb = np.asarray(sb).astype(np.int64)
    sib = np.asarray(sib).astype(np.int64)
    S = sib.shape[0]
    G = cfg.NCORES * cfg.G_SH
    seg_cnt = np.bincount(sb, minlength=S)
    g_cnt = np.bincount(sib, minlength=G)
    h_bf16 = np.asarray(h_node).astype(BF16)
    return [
        _plan_core(cfg, c, h_bf16, sb, sib, seg_cnt, g_cnt)
        for c in range(cfg.NCORES)
    ]


# ---------------------------------------------------------------------------
# bass program
# ---------------------------------------------------------------------------


def build_bass(cfg):
    import concourse.mybir as mybir
    import concourse.tile as tile
    from concourse import bacc

    f32 = mybir.dt.float32
    bf16 = mybir.dt.bfloat16
    AF = mybir.ActivationFunctionType
    OP = mybir.AluOpType
    D, C, W = cfg.D, cfg.C, cfg.W
    HALVES = P // W          # psum chunks per seg tile
    TPW = cfg.T2 * P         # padded segs per graph chunk

    nc = bacc.Bacc("TRN2", target_bir_lowering=False, debug=False)

    def din(name, shape, dt=f32):
        return nc.dram_tensor(name, shape, dt, kind="ExternalInput").ap()

    hp_d = din("hp", [cfg.NCH, P, cfg.T1 * D], bf16)
    rel_d = din("rel", [P, cfg.NSLOT], bf16)
    invs_d = din("invs", [P, cfg.NS])
    rel2_d = din("rel2", [P, cfg.NS], bf16)
    invg_d = din("invg", [P, cfg.NGC])
    gidb_d = din("gidb", [P, cfg.SMAXP], bf16)
    iota_d = din("iota", [P, P], bf16)
    iotag_d = din("iotag", [P, cfg.NGC])
    ident_d = din("ident", [P, P], bf16)

    w_d = {}
    for l in range(2):
        w_d[f"W{l}"] = din(f"W{l}", [D, C])
        w_d[f"Ws{l}"] = din(f"Ws{l}", [D, C])
        w_d[f"b{l}"] = din(f"b{l}", [C])
        w_d[f"bs{l}"] = din(f"bs{l}", [C])
    w_d["Wf1"] = din("Wf1", [C, 2 * C])
    w_d["bf1"] = din("bf1", [2 * C])
    w_d["Wf2"] = din("Wf2", [2 * C, 10])
    w_d["bf2"] = din("bf2", [10])

    out_d = nc.dram_tensor("out", [10, cfg.G_SH], f32, kind="ExternalOutput").ap()

    with tile.TileContext(nc) as tc:
        with (
            tc.tile_pool(name="persist", bufs=1) as pp,
            tc.tile_pool(name="stream", bufs=2) as sp,
            tc.tile_pool(name="small", bufs=2) as mp,
            tc.tile_pool(name="psum_acc", bufs=4, space="PSUM") as pacc,
            tc.tile_pool(name="psum_tr", bufs=2, space="PSUM") as ptr,
            tc.tile_pool(name="psum_wide", bufs=2, space="PSUM") as pwide,
        ):
            # ---- constants / weights to SBUF -------------------------------
            def load(ap_dram, shape, dt):
                t = pp.tile(shape, dt, tag=f"ld_{ap_dram.tensor.name}")
                nc.sync.dma_start(t[:], ap_dram)
                return t

            iota = load(iota_d, [P, P], bf16)
            iotag = load(iotag_d, [P, cfg.NGC], f32)
            ident = load(ident_d, [P, P], bf16)
            rel = load(rel_d, [P, cfg.NSLOT], bf16)
            invs = load(invs_d, [P, cfg.NS], f32)
            rel2 = load(rel2_d, [P, cfg.NS], bf16)
            invg = load(invg_d, [P, cfg.NGC], f32)

            def cast_bf16(name, tf, shape):
                tb = pp.tile(shape, bf16, tag=f"bf_{name}")
                nc.vector.tensor_copy(tb[:], tf[:])
                return tb

            Wl, Ws, bsum = [], [], []
            for l in range(2):
                Wl.append(cast_bf16(
                    f"W{l}", load(w_d[f"W{l}"], [D, C], f32), [D, C]))
                Ws.append(cast_bf16(
                    f"Ws{l}", load(w_d[f"Ws{l}"], [D, C], f32), [D, C]))
                b_t = load(w_d[f"b{l}"].unsqueeze(1), [P, 1], f32)
                bs_t = load(w_d[f"bs{l}"].unsqueeze(1), [P, 1], f32)
                s = pp.tile([P, 1], f32, tag=f"bsum{l}")
                nc.vector.tensor_tensor(s[:], b_t[:], bs_t[:], op=OP.add)
                bsum.append(s)
            Wf1 = cast_bf16("Wf1", load(w_d["Wf1"], [C, 2 * C], f32),
                            [C, 2 * C])
            Wf2 = cast_bf16(
                "Wf2",
                load(w_d["Wf2"].rearrange("(h p) t -> p h t", h=2),
                     [P, 2, 10], f32),
                [P, 2, 10])
            bf1 = load(w_d["bf1"].rearrange("(h p) -> p h", h=2), [P, 2], f32)
            bf2_t = pp.tile([P, 1], f32, tag="ld_bf2")
            nc.sync.dma_start(bf2_t[:10, :], w_d["bf2"].unsqueeze(1))

            # persistent activations: per graph chunk [seg_p, (t2, d)]
            hs_a = [pp.tile([P, cfg.T2, D], bf16, tag=f"hsa{gc}", name=f"hsa{gc}")
                    for gc in range(cfg.NGC)]
            hs_b = [pp.tile([P, cfg.T2, D], bf16, tag=f"hsb{gc}", name=f"hsb{gc}")
                    for gc in range(cfg.NGC)]
            ohg = [pp.tile([P, cfg.T2, P], bf16, tag=f"ohg{gc}", name=f"ohg{gc}")
                   for gc in range(cfg.NGC)]

            # ---- graph-level one-hots (built once, reused) -----------------
            for gc in range(cfg.NGC):
                r2b = rel2[:, gc * cfg.T2:(gc + 1) * cfg.T2] \
                    .unsqueeze(2).to_broadcast([P, cfg.T2, P])
                iob = iota[:].unsqueeze(1).to_broadcast([P, cfg.T2, P])
                nc.vector.tensor_tensor(ohg[gc][:], r2b, iob, op=OP.is_equal)

            # ---- phase 1: node -> subgraph mean ----------------------------
            for cch in range(cfg.NCH):
                k, hh = cch // HALVES, cch % HALVES     # seg tile, half
                gc, t2 = k // cfg.T2, k % cfg.T2
                hpt = sp.tile([P, cfg.T1 * D], bf16, tag="hp", bufs=3)
                nc.sync.dma_start(hpt[:], hp_d[cch])
                oh = sp.tile([P, cfg.T1, W], bf16, tag="oh", bufs=3)
                rb = rel[:, cch * cfg.T1:(cch + 1) * cfg.T1] \
                    .unsqueeze(2).to_broadcast([P, cfg.T1, W])
                iob = iota[:, :W].unsqueeze(1).to_broadcast([P, cfg.T1, W])
                nc.vector.tensor_tensor(oh[:], rb, iob, op=OP.is_equal)
                ps = pacc.tile([P, D], f32, tag="acc")
                for t in range(cfg.T1):
                    nc.tensor.matmul(
                        ps[:W, :], lhsT=oh[:, t, :],
                        rhs=hpt[:, t * D:(t + 1) * D],
                        start=(t == 0), stop=(t == cfg.T1 - 1))
                nc.scalar.activation(
                    hs_a[gc][hh * W:(hh + 1) * W, t2, :], ps[:W, :], AF.Copy,
                    scale=invs[hh * W:(hh + 1) * W, k:k + 1])

            # ---- DS layers -------------------------------------------------
            hs_in, hs_out = hs_a, hs_b
            for l in range(2):
                # graph means -> transposed [d, g] table
                gmT = mp.tile([P, cfg.NGC * P], bf16, tag="gmT")
                for gc in range(cfg.NGC):
                    psg = pacc.tile([P, D], f32, tag="acc")
                    for t2 in range(cfg.T2):
                        nc.tensor.matmul(
                            psg[:], lhsT=ohg[gc][:, t2, :],
                            rhs=hs_in[gc][:, t2, :],
                            start=(t2 == 0), stop=(t2 == cfg.T2 - 1))
                    gm = mp.tile([P, D], bf16, tag="gm")
                    nc.scalar.activation(gm[:], psg[:], AF.Copy,
                                         scale=invg[:, gc:gc + 1])
                    ptt = ptr.tile([P, P], bf16, tag="tr")
                    nc.tensor.transpose(ptt[:], gm[:], ident[:])
                    nc.scalar.activation(gmT[:, gc * P:(gc + 1) * P], ptt[:], AF.Copy)

                # x2 = gmean @ Ws + (b + bs), row-major in SBUF
                x2ps = pwide.tile([P, cfg.NGC * P], f32, tag="wide")
                nc.tensor.matmul(x2ps[:], lhsT=Ws[l][:], rhs=gmT[:],
                                 start=True, stop=True)
                x2T = mp.tile([P, cfg.NGC * P], bf16, tag="x2T")
                nc.scalar.activation(x2T[:], x2ps[:], AF.Identity,
                                     bias=bsum[l][:])
                x2rm = mp.tile([P, cfg.NGC, C], bf16, tag="x2rm")
                for gc in range(cfg.NGC):
                    ptt = ptr.tile([P, P], bf16, tag="tr")
                    nc.tensor.transpose(ptt[:], x2T[:, gc * P:(gc + 1) * P],
                                        ident[:])
                    nc.scalar.activation(x2rm[:, gc, :], ptt[:], AF.Copy)

                for gc in range(cfg.NGC):
                    # transposed graph one-hot [g, seg] for the x2 broadcast
                    gsl = sp.tile([P, TPW], bf16, tag="gsl")
                    nc.sync.dma_start(
                        gsl[:], gidb_d[:, gc * TPW:(gc + 1) * TPW])
                    ohgT = gsl
                    nc.vector.tensor_scalar(
                        ohgT[:], gsl[:], iotag[:, gc:gc + 1], None,
                        op0=OP.is_equal)
                    for s0 in range(0, cfg.T2, cfg.SWATH):
                        sl = min(cfg.SWATH, cfg.T2 - s0)
                        comb = mp.tile([P, cfg.SWATH, C], f32, tag="comb")
                        for j in range(sl):
                            t2 = s0 + j
                            ptt = ptr.tile([P, P], bf16, tag="tr")
                            nc.tensor.transpose(ptt[:], hs_in[gc][:, t2, :],
                                                ident[:])
                            hT = mp.tile([P, P], bf16, tag="hT")
                            nc.scalar.activation(hT[:], ptt[:], AF.Copy)
                            x1p = pacc.tile([P, C], f32, tag="acc")
                            nc.tensor.matmul(x1p[:], lhsT=hT[:], rhs=Wl[l][:],
                                             start=True, stop=False)
                            nc.tensor.matmul(
                                x1p[:], lhsT=ohgT[:, t2 * P:(t2 + 1) * P],
                                rhs=x2rm[:, gc, :], start=False, stop=True)
                            nc.scalar.activation(comb[:, j, :], x1p[:],
                                                 AF.Copy)
                        # elu(comb) -> hs_out, flattened over the swath
                        cf = comb[:, :sl, :].rearrange("p a b -> p (a b)")
                        F = sl * C
                        neg = mp.tile([P, cfg.SWATH * C], f32, tag="neg")
                        nc.vector.tensor_scalar_min(neg[:, :F], cf, 0.0)
                        ex = neg
                        nc.scalar.activation(ex[:, :F], neg[:, :F], AF.Exp)
                        nc.vector.tensor_scalar(
                            cf, cf, 0.0, -1.0, op0=OP.max, op1=OP.add)
                        ho = hs_out[gc][:, s0:s0 + sl, :]
                        nc.vector.tensor_tensor(
                            ho.rearrange("p a b -> p (a b)"), ex[:, :F], cf,
                            op=OP.add)
                hs_in, hs_out = hs_out, hs_in

            # ---- head ------------------------------------------------------
            hgT = mp.tile([P, cfg.NGC * P], bf16, tag="hgT")
            for gc in range(cfg.NGC):
                psg = pacc.tile([P, D], f32, tag="acc")
                for t2 in range(cfg.T2):
                    nc.tensor.matmul(
                        psg[:], lhsT=ohg[gc][:, t2, :],
                        rhs=hs_in[gc][:, t2, :],
                        start=(t2 == 0), stop=(t2 == cfg.T2 - 1))
                gm = mp.tile([P, D], bf16, tag="gm")
                nc.scalar.activation(gm[:], psg[:], AF.Copy,
                                     scale=invg[:, gc:gc + 1])
                ptt = ptr.tile([P, P], bf16, tag="tr")
                nc.tensor.transpose(ptt[:], gm[:], ident[:])
                nc.scalar.activation(hgT[:, gc * P:(gc + 1) * P], ptt[:], AF.Copy)

            y1 = []
            for h in range(2):
                yps = pwide.tile([P, cfg.NGC * P], f32, tag="wide")
                nc.tensor.matmul(yps[:], lhsT=Wf1[:, h * C:(h + 1) * C],
                                 rhs=hgT[:], start=True, stop=True)
                y1t = mp.tile([P, cfg.NGC * P], bf16, tag=f"y1_{h}")
                nc.scalar.activation(y1t[:], yps[:], AF.Relu,
                                     bias=bf1[:, h:h + 1])
                y1.append(y1t)
            y2ps = pwide.tile([P, cfg.NGC * P], f32, tag="wide")
            for h in range(2):
                nc.tensor.matmul(y2ps[:10, :], lhsT=Wf2[:, h, :],
                                 rhs=y1[h][:], start=(h == 0), stop=(h == 1))
            yout = mp.tile([P, cfg.NGC * P], f32, tag="yout")
            nc.scalar.activation(yout[:10, :], y2ps[:10, :], AF.Identity,
                                 bias=bf2_t[:10, :])
            nc.sync.dma_start(out_d[:], yout[:10, :])

    nc.compile()
    return nc


# ---------------------------------------------------------------------------
# entry point
# ---------------------------------------------------------------------------

_CACHED = {}


def _get_nc(cfg):
    key = (cfg.W, cfg.T1, cfg.T2, cfg.NGC, cfg.G_SH, cfg.NCORES, cfg.SWATH)
    if key not in _CACHED:
        _CACHED[key] = build_bass(cfg)
    return _CACHED[key]


def make_in_maps(cfg, inputs):
    plans = plan(cfg, inputs["h_node"], inputs["subgraph_batch"],
                 inputs["subgraph_idx_batch"])
    iota = np.broadcast_to(
        np.arange(P, dtype=np.float32), (P, P)).astype(BF16)
    iotag = np.broadcast_to(
        np.arange(P, dtype=np.float32)[:, None], (P, cfg.NGC)).copy()
    ident = np.eye(P, dtype=BF16)
    shared = {
        "iota": iota,
        "iotag": iotag,
        "ident": ident,
        "W0": np.asarray(inputs["W_fc0"], np.float32),
        "Ws0": np.asarray(inputs["W_sum0"], np.float32),
        "b0": np.asarray(inputs["b_fc0"], np.float32),
        "bs0": np.asarray(inputs["b_sum0"], np.float32),
        "W1": np.asarray(inputs["W_fc1"], np.float32),
        "Ws1": np.asarray(inputs["W_sum1"], np.float32),
        "b1": np.asarray(inputs["b_fc1"], np.float32),
        "bs1": np.asarray(inputs["b_sum1"], np.float32),
        "Wf1": np.asarray(inputs["Wf1"], np.float32),
        "bf1": np.asarray(inputs["bf1"], np.float32),
        "Wf2": np.asarray(inputs["Wf2"], np.float32),
        "bf2": np.asarray(inputs["bf2"], np.float32),
    }
    return [dict(shared, **p) for p in plans]


def run(cfg, inputs, trace=False):
    from concourse.bass_utils import run_bass_kernel_spmd

    in_maps = make_in_maps(cfg, inputs)
    nc = _get_nc(cfg)
    res = run_bass_kernel_spmd(nc, in_maps, list(range(cfg.NCORES)),
                               trace=trace)
    outs = [np.asarray(res.results[c]["out"]).T for c in range(cfg.NCORES)]
    out = np.concatenate(outs, axis=0).astype(np.float32)
    return out, res


def kernel(**inputs) -> np.ndarray:
    out, _ = run(FULL, inputs)
    return out


# revision 14
# speedup vs baseline: 1.0317x; 1.0317x over previous
"""Trainium2 Bass kernel for nn_DSnetwork (gnn_message_passing).

Reference computation (S=131072 subgraphs, G=4096 graphs, N=2M nodes, D=128):
  h_sub  = segment_mean(h_node, subgraph_batch, S)            # [S,128]
  2x DS layers:
    x1 = h_sub @ W + b
    x2 = segment_mean(h_sub, subgraph_idx_batch, G) @ Ws + bs
    h_sub = elu(x1 + x2[subgraph_idx_batch])
  h_graph = segment_mean(h_sub, subgraph_idx_batch, G)
  out = relu(h_graph @ Wf1 + bf1) @ Wf2 + bf2                 # [G,10]

Distribution: data-parallel over contiguous graph ranges (512 graphs per
core, 8 cores).  Indices are sorted, so each core owns contiguous slices
of subgraphs and nodes.  Segment sums run on TensorE as one-hot matmuls
(one-hots built on VectorE from host-precomputed relative ids); the
graph->subgraph broadcast is a transposed-one-hot matmul accumulated
directly into the x1 PSUM tile.  Matmuls are bf16 with fp32 PSUM
accumulation; mean scaling is exact fp32 on ScalarE.

Host-side work is pure index preprocessing and data staging: sharding,
padded placement of subgraphs/nodes into static tiles, relative one-hot
ids, 1/count scale vectors, and dtype casts.
"""

from dataclasses import dataclass

import ml_dtypes
import numpy as np

BF16 = ml_dtypes.bfloat16
P = 128


@dataclass(frozen=True)
class Cfg:
    D: int = 128          # node feature dim
    C: int = 128          # hidden dim
    NCORES: int = 8
    G_SH: int = 512       # graphs per core
    NGC: int = 4          # graph chunks of 128 graphs per core
    T2: int = 34          # seg tiles (128 segs) per graph chunk, padded
    W: int = 128          # phase-1 one-hot window (segs per psum chunk)
    T1: int = 17          # node tile slots (128 nodes) per W-seg chunk
    SWATH: int = 8        # seg tiles per elu swath (ragged tail ok)

    @property
    def NS(self):         # seg tiles per core (padded axis)
        return self.NGC * self.T2

    @property
    def SMAXP(self):      # padded segs per core
        return self.NS * P

    @property
    def NCH(self):        # phase-1 chunks per core
        return self.SMAXP // self.W

    @property
    def NSLOT(self):      # node tile slots per core
        return self.NCH * self.T1


FULL = Cfg()

# ---------------------------------------------------------------------------
# host-side planner: shard + metadata layout
# ---------------------------------------------------------------------------


def _plan_core(cfg, core, h_node_bf16, sb, sib, seg_cnt, g_cnt):
    g0 = core * cfg.G_SH
    W = cfg.W

    hp = np.zeros((cfg.NSLOT * P, cfg.D), dtype=BF16)
    rel = np.full((P, cfg.NSLOT), -1.0, dtype=BF16)
    invs = np.zeros((P, cfg.NS), dtype=np.float32)
    rel2 = np.full((P, cfg.NS), -1.0, dtype=BF16)
    invg = np.zeros((P, cfg.NGC), dtype=np.float32)

    # chunk-local graph id per padded seg (-1 pad; fits bf16 exactly)
    gid_pad = np.full(cfg.SMAXP, -1.0, dtype=np.float64)

    for gc in range(cfg.NGC):
        glo = g0 + gc * P
        ghi = glo + P
        a = int(np.searchsorted(sib, glo))
        b = int(np.searchsorted(sib, ghi))
        nseg = b - a
        assert nseg <= cfg.T2 * P, f"T2 too small: {nseg} > {cfg.T2 * P}"
        base_tile = gc * cfg.T2          # first seg tile of this graph chunk

        gl = (sib[a:b] - glo).astype(np.int64)          # in [0,128)
        pad_pos = base_tile * P
        gid_pad[pad_pos:pad_pos + nseg] = gl

        r2 = np.full(cfg.T2 * P, -1.0, dtype=BF16)
        r2[:nseg] = gl.astype(BF16)
        rel2[:, base_tile:base_tile + cfg.T2] = r2.reshape(cfg.T2, P).T

        ivs = np.zeros(cfg.T2 * P, dtype=np.float32)
        ivs[:nseg] = 1.0 / np.maximum(seg_cnt[a:b], 1).astype(np.float32)
        invs[:, base_tile:base_tile + cfg.T2] = ivs.reshape(cfg.T2, P).T

        invg[:, gc] = 1.0 / np.maximum(g_cnt[glo:ghi], 1).astype(np.float32)

        # node packing: phase-1 chunks of W padded segs
        seg_starts = np.searchsorted(sb, np.arange(a, b + 1))
        nch_per_gc = cfg.T2 * P // W
        for cc in range(nch_per_gc):
            c = (base_tile * P) // W + cc            # global chunk index
            slo = cc * W
            shi = min(slo + W, nseg)
            if slo >= nseg:
                continue
            nlo = int(seg_starts[slo])
            nhi = int(seg_starts[shi])
            nn = nhi - nlo
            assert nn <= cfg.T1 * P, f"T1 too small: {nn} > {cfg.T1 * P}"
            if nn == 0:
                continue
            dst = c * cfg.T1 * P
            hp[dst:dst + nn] = h_node_bf16[nlo:nhi]
            rr = (sb[nlo:nhi] - (a + slo)).astype(BF16)
            rfull = np.full(cfg.T1 * P, -1.0, dtype=BF16)
            rfull[:nn] = rr
            rel[:, c * cfg.T1:(c + 1) * cfg.T1] = rfull.reshape(cfg.T1, P).T

    # device layout: hp_dram[c, p, t*D+d] = node row (c*T1*P + t*P + p)
    hp_dev = np.ascontiguousarray(
        hp.reshape(cfg.NCH, cfg.T1, P, cfg.D).transpose(0, 2, 1, 3)
    ).reshape(cfg.NCH, P, cfg.T1 * cfg.D)

    gidb = np.broadcast_to(gid_pad.astype(BF16), (P, cfg.SMAXP)).copy()

    return {
        "hp": hp_dev,
        "rel": rel,
        "invs": invs,
        "rel2": rel2,
        "invg": invg,
        "gidb": gidb,
    }


def plan(cfg, h_node, sb, sib):
    sb = np.asarray(sb).astype(np.int64)
    sib = np.asarray(sib).astype(np.int64)
    S = sib.shape[0]
    G = cfg.NCORES * cfg.G_SH
    seg_cnt = np.bincount(sb, minlength=S)
    g_cnt = np.bincount(sib, minlength=G)
    h_bf16 = np.asarray(h_node).astype(BF16)
    return [
        _plan_core(cfg, c, h_bf16, sb, sib, seg_cnt, g_cnt)
        for c in range(cfg.NCORES)
    ]


# ---------------------------------------------------------------------------
# bass program
# ---------------------------------------------------------------------------


def build_bass(cfg):
    import concourse.mybir as mybir
    import concourse.tile as tile
    from concourse import bacc

    f32 = mybir.dt.float32
    bf16 = mybir.dt.bfloat16
    AF = mybir.ActivationFunctionType
    OP = mybir.AluOpType
    D, C, W = cfg.D, cfg.C, cfg.W
    HALVES = P // W          # psum chunks per seg tile
    TPW = cfg.T2 * P         # padded segs per graph chunk

    nc = bacc.Bacc("TRN2", target_bir_lowering=False, debug=False)

    def din(name, shape, dt=f32):
        return nc.dram_tensor(name, shape, dt, kind="ExternalInput").ap()

    hp_d = din("hp", [cfg.NCH, P, cfg.T1 * D], bf16)
    rel_d = din("rel", [P, cfg.NSLOT], bf16)
    invs_d = din("invs", [P, cfg.NS])
    rel2_d = din("rel2", [P, cfg.NS], bf16)
    invg_d = din("invg", [P, cfg.NGC])
    gidb_d = din("gidb", [P, cfg.SMAXP], bf16)
    iota_d = din("iota", [P, P], bf16)
    iotag_d = din("iotag", [P, cfg.NGC])
    ident_d = din("ident", [P, P], bf16)

    w_d = {}
    for l in range(2):
        w_d[f"W{l}"] = din(f"W{l}", [D, C])
        w_d[f"Ws{l}"] = din(f"Ws{l}", [D, C])
        w_d[f"b{l}"] = din(f"b{l}", [C])
        w_d[f"bs{l}"] = din(f"bs{l}", [C])
    w_d["Wf1"] = din("Wf1", [C, 2 * C])
    w_d["bf1"] = din("bf1", [2 * C])
    w_d["Wf2"] = din("Wf2", [2 * C, 10])
    w_d["bf2"] = din("bf2", [10])

    out_d = nc.dram_tensor("out", [10, cfg.G_SH], f32, kind="ExternalOutput").ap()

    with tile.TileContext(nc) as tc:
        with (
            tc.tile_pool(name="persist", bufs=1) as pp,
            tc.tile_pool(name="stream", bufs=2) as sp,
            tc.tile_pool(name="small", bufs=2) as mp,
            tc.tile_pool(name="psum_acc", bufs=4, space="PSUM") as pacc,
            tc.tile_pool(name="psum_tr", bufs=2, space="PSUM") as ptr,
            tc.tile_pool(name="psum_wide", bufs=2, space="PSUM") as pwide,
        ):
            # ---- constants / weights to SBUF -------------------------------
            def load(ap_dram, shape, dt):
                t = pp.tile(shape, dt, tag=f"ld_{ap_dram.tensor.name}")
                nc.sync.dma_start(t[:], ap_dram)
                return t

            iota = load(iota_d, [P, P], bf16)
            iotag = load(iotag_d, [P, cfg.NGC], f32)
            ident = load(ident_d, [P, P], bf16)
            rel = load(rel_d, [P, cfg.NSLOT], bf16)
            invs = load(invs_d, [P, cfg.NS], f32)
            rel2 = load(rel2_d, [P, cfg.NS], bf16)
            invg = load(invg_d, [P, cfg.NGC], f32)

            def cast_bf16(name, tf, shape):
                tb = pp.tile(shape, bf16, tag=f"bf_{name}")
                nc.vector.tensor_copy(tb[:], tf[:])
                return tb

            Wl, Ws, bsum = [], [], []
            for l in range(2):
                Wl.append(cast_bf16(
                    f"W{l}", load(w_d[f"W{l}"], [D, C], f32), [D, C]))
                Ws.append(cast_bf16(
                    f"Ws{l}", load(w_d[f"Ws{l}"], [D, C], f32), [D, C]))
                b_t = load(w_d[f"b{l}"].unsqueeze(1), [P, 1], f32)
                bs_t = load(w_d[f"bs{l}"].unsqueeze(1), [P, 1], f32)
                s = pp.tile([P, 1], f32, tag=f"bsum{l}")
                nc.vector.tensor_tensor(s[:], b_t[:], bs_t[:], op=OP.add)
                bsum.append(s)
            Wf1 = cast_bf16("Wf1", load(w_d["Wf1"], [C, 2 * C], f32),
                            [C, 2 * C])
            Wf2 = cast_bf16(
                "Wf2",
                load(w_d["Wf2"].rearrange("(h p) t -> p h t", h=2),
                     [P, 2, 10], f32),
                [P, 2, 10])
            bf1 = load(w_d["bf1"].rearrange("(h p) -> p h", h=2), [P, 2], f32)
            bf2_t = pp.tile([P, 1], f32, tag="ld_bf2")
            nc.sync.dma_start(bf2_t[:10, :], w_d["bf2"].unsqueeze(1))

            # persistent activations: per graph chunk [seg_p, (t2, d)]
            hs_a = [pp.tile([P, cfg.T2, D], bf16, tag=f"hsa{gc}", name=f"hsa{gc}")
                    for gc in range(cfg.NGC)]
            hs_b = [pp.tile([P, cfg.T2, D], bf16, tag=f"hsb{gc}", name=f"hsb{gc}")
                    for gc in range(cfg.NGC)]
            ohg = [pp.tile([P, cfg.T2, P], bf16, tag=f"ohg{gc}", name=f"ohg{gc}")
                   for gc in range(cfg.NGC)]

            # ---- graph-level one-hots (built once, reused) -----------------
            for gc in range(cfg.NGC):
                r2b = rel2[:, gc * cfg.T2:(gc + 1) * cfg.T2] \
                    .unsqueeze(2).to_broadcast([P, cfg.T2, P])
                iob = iota[:].unsqueeze(1).to_broadcast([P, cfg.T2, P])
                nc.vector.tensor_tensor(ohg[gc][:], r2b, iob, op=OP.is_equal)

            # ---- phase 1: node -> subgraph mean ----------------------------
            for cch in range(cfg.NCH):
                k, hh = cch // HALVES, cch % HALVES     # seg tile, half
                gc, t2 = k // cfg.T2, k % cfg.T2
                hpt = sp.tile([P, cfg.T1 * D], bf16, tag="hp", bufs=3)
                nc.sync.dma_start(hpt[:], hp_d[cch])
                oh = sp.tile([P, cfg.T1, W], bf16, tag="oh", bufs=3)
                rb = rel[:, cch * cfg.T1:(cch + 1) * cfg.T1] \
                    .unsqueeze(2).to_broadcast([P, cfg.T1, W])
                iob = iota[:, :W].unsqueeze(1).to_broadcast([P, cfg.T1, W])
                nc.vector.tensor_tensor(oh[:], rb, iob, op=OP.is_equal)
                ps = pacc.tile([P, D], f32, tag="acc")
                for t in range(cfg.T1):
                    nc.tensor.matmul(
                        ps[:W, :], lhsT=oh[:, t, :],
                        rhs=hpt[:, t * D:(t + 1) * D],
                        start=(t == 0), stop=(t == cfg.T1 - 1))
                nc.scalar.activation(
                    hs_a[gc][hh * W:(hh + 1) * W, t2, :], ps[:W, :], AF.Copy,
                    scale=invs[hh * W:(hh + 1) * W, k:k + 1])

            # ---- DS layers -------------------------------------------------
            hs_in, hs_out = hs_a, hs_b
            for l in range(2):
                # graph means -> transposed [d, g] table
                gmT = mp.tile([P, cfg.NGC * P], bf16, tag="gmT")
                for gc in range(cfg.NGC):
                    psg = pacc.tile([P, D], f32, tag="acc")
                    for t2 in range(cfg.T2):
                        nc.tensor.matmul(
                            psg[:], lhsT=ohg[gc][:, t2, :],
                            rhs=hs_in[gc][:, t2, :],
                            start=(t2 == 0), stop=(t2 == cfg.T2 - 1))
                    gm = mp.tile([P, D], bf16, tag="gm")
                    nc.scalar.activation(gm[:], psg[:], AF.Copy,
                                         scale=invg[:, gc:gc + 1])
                    ptt = ptr.tile([P, P], bf16, tag="tr")
                    nc.tensor.transpose(ptt[:], gm[:], ident[:])
                    nc.scalar.activation(gmT[:, gc * P:(gc + 1) * P], ptt[:], AF.Copy)

                # x2 = gmean @ Ws + (b + bs), row-major in SBUF
                x2ps = pwide.tile([P, cfg.NGC * P], f32, tag="wide")
                nc.tensor.matmul(x2ps[:], lhsT=Ws[l][:], rhs=gmT[:],
                                 start=True, stop=True)
                x2T = mp.tile([P, cfg.NGC * P], bf16, tag="x2T")
                nc.scalar.activation(x2T[:], x2ps[:], AF.Identity,
                                     bias=bsum[l][:])
                x2rm = mp.tile([P, cfg.NGC, C], bf16, tag="x2rm")
                for gc in range(cfg.NGC):
                    ptt = ptr.tile([P, P], bf16, tag="tr")
                    nc.tensor.transpose(ptt[:], x2T[:, gc * P:(gc + 1) * P],
                                        ident[:])
                    nc.scalar.activation(x2rm[:, gc, :], ptt[:], AF.Copy)

                for gc in range(cfg.NGC):
                    # transposed graph one-hot [g, seg] for the x2 broadcast
                    gsl = sp.tile([P, TPW], bf16, tag="gsl")
                    nc.sync.dma_start(
                        gsl[:], gidb_d[:, gc * TPW:(gc + 1) * TPW])
                    ohgT = gsl
                    nc.vector.tensor_scalar(
                        ohgT[:], gsl[:], iotag[:, gc:gc + 1], None,
                        op0=OP.is_equal)
                    for s0 in range(0, cfg.T2, cfg.SWATH):
                        sl = min(cfg.SWATH, cfg.T2 - s0)
                        comb = mp.tile([P, cfg.SWATH, C], f32, tag="comb")
                        for j in range(sl):
                            t2 = s0 + j
                            ptt = ptr.tile([P, P], bf16, tag="tr")
                            nc.tensor.transpose(ptt[:], hs_in[gc][:, t2, :],
                                                ident[:])
                            hT = mp.tile([P, P], bf16, tag="hT")
                            nc.vector.tensor_copy(hT[:], ptt[:])
                            x1p = pacc.tile([P, C], f32, tag="acc")
                            nc.tensor.matmul(x1p[:], lhsT=hT[:], rhs=Wl[l][:],
                                             start=True, stop=False)
                            nc.tensor.matmul(
                                x1p[:], lhsT=ohgT[:, t2 * P:(t2 + 1) * P],
                                rhs=x2rm[:, gc, :], start=False, stop=True)
                            nc.scalar.activation(comb[:, j, :], x1p[:],
                                                 AF.Copy)
                        # elu(x) = exp(min(x,0)) - 1 + relu(x)
                        cf = comb[:, :sl, :].rearrange("p a b -> p (a b)")
                        F = sl * C
                        u = mp.tile([P, cfg.SWATH * C], f32, tag="neg")
                        nc.scalar.activation(u[:, :F], cf, AF.Relu,
                                             scale=-1.0)      # -min(x,0)
                        nc.scalar.activation(u[:, :F], u[:, :F], AF.Exp,
                                             scale=-1.0)      # exp(min(x,0))
                        r = mp.tile([P, cfg.SWATH * C], f32, tag="ex")
                        nc.scalar.activation(r[:, :F], cf, AF.Relu)
                        ho = hs_out[gc][:, s0:s0 + sl, :]
                        nc.vector.scalar_tensor_tensor(
                            ho.rearrange("p a b -> p (a b)"), u[:, :F], -1.0,
                            r[:, :F], op0=OP.add, op1=OP.add)
                hs_in, hs_out = hs_out, hs_in

            # ---- head ------------------------------------------------------
            hgT = mp.tile([P, cfg.NGC * P], bf16, tag="hgT")
            for gc in range(cfg.NGC):
                psg = pacc.tile([P, D], f32, tag="acc")
                for t2 in range(cfg.T2):
                    nc.tensor.matmul(
                        psg[:], lhsT=ohg[gc][:, t2, :],
                        rhs=hs_in[gc][:, t2, :],
                        start=(t2 == 0), stop=(t2 == cfg.T2 - 1))
                gm = mp.tile([P, D], bf16, tag="gm")
                nc.scalar.activation(gm[:], psg[:], AF.Copy,
                                     scale=invg[:, gc:gc + 1])
                ptt = ptr.tile([P, P], bf16, tag="tr")
                nc.tensor.transpose(ptt[:], gm[:], ident[:])
                nc.scalar.activation(hgT[:, gc * P:(gc + 1) * P], ptt[:], AF.Copy)

            y1 = []
            for h in range(2):
                yps = pwide.tile([P, cfg.NGC * P], f32, tag="wide")
                nc.tensor.matmul(yps[:], lhsT=Wf1[:, h * C:(h + 1) * C],
                                 rhs=hgT[:], start=True, stop=True)
                y1t = mp.tile([P, cfg.NGC * P], bf16, tag=f"y1_{h}")
                nc.scalar.activation(y1t[:], yps[:], AF.Relu,
                                     bias=bf1[:, h:h + 1])
                y1.append(y1t)
            y2ps = pwide.tile([P, cfg.NGC * P], f32, tag="wide")
            for h in range(2):
                nc.tensor.matmul(y2ps[:10, :], lhsT=Wf2[:, h, :],
                                 rhs=y1[h][:], start=(h == 0), stop=(h == 1))
            yout = mp.tile([P, cfg.NGC * P], f32, tag="yout")
            nc.scalar.activation(yout[:10, :], y2ps[:10, :], AF.Identity,
                                 bias=bf2_t[:10, :])
            nc.sync.dma_start(out_d[:], yout[:10, :])

    nc.compile()
    return nc


# ---------------------------------------------------------------------------
# entry point
# ---------------------------------------------------------------------------

_CACHED = {}


def _get_nc(cfg):
    key = (cfg.W, cfg.T1, cfg.T2, cfg.NGC, cfg.G_SH, cfg.NCORES, cfg.SWATH)
    if key not in _CACHED:
        _CACHED[key] = build_bass(cfg)
    return _CACHED[key]


def make_in_maps(cfg, inputs):
    plans = plan(cfg, inputs["h_node"], inputs["subgraph_batch"],
                 inputs["subgraph_idx_batch"])
    iota = np.broadcast_to(
        np.arange(P, dtype=np.float32), (P, P)).astype(BF16)
    iotag = np.broadcast_to(
        np.arange(P, dtype=np.float32)[:, None], (P, cfg.NGC)).copy()
    ident = np.eye(P, dtype=BF16)
    shared = {
        "iota": iota,
        "iotag": iotag,
        "ident": ident,
        "W0": np.asarray(inputs["W_fc0"], np.float32),
        "Ws0": np.asarray(inputs["W_sum0"], np.float32),
        "b0": np.asarray(inputs["b_fc0"], np.float32),
        "bs0": np.asarray(inputs["b_sum0"], np.float32),
        "W1": np.asarray(inputs["W_fc1"], np.float32),
        "Ws1": np.asarray(inputs["W_sum1"], np.float32),
        "b1": np.asarray(inputs["b_fc1"], np.float32),
        "bs1": np.asarray(inputs["b_sum1"], np.float32),
        "Wf1": np.asarray(inputs["Wf1"], np.float32),
        "bf1": np.asarray(inputs["bf1"], np.float32),
        "Wf2": np.asarray(inputs["Wf2"], np.float32),
        "bf2": np.asarray(inputs["bf2"], np.float32),
    }
    return [dict(shared, **p) for p in plans]


def run(cfg, inputs, trace=False):
    from concourse.bass_utils import run_bass_kernel_spmd

    in_maps = make_in_maps(cfg, inputs)
    nc = _get_nc(cfg)
    res = run_bass_kernel_spmd(nc, in_maps, list(range(cfg.NCORES)),
                               trace=trace)
    outs = [np.asarray(res.results[c]["out"]).T for c in range(cfg.NCORES)]
    out = np.concatenate(outs, axis=0).astype(np.float32)
    return out, res


def kernel(**inputs) -> np.ndarray:
    out, _ = run(FULL, inputs)
    return out
